# revision 38
# baseline (speedup 1.0000x reference)
"""Bass/Tile SPMD kernel for nn_CEN_BRL_22763326668900 on 8 trn2 NeuronCores.

Reference computation:
  phi = relu(ctx @ w1.T + b1) @ w2.T + b2            [4096, 256]
  pre = S.T @ Wa.T                                   [2048, 256]
  16 LSTM steps with x_t rank-1:
      x_0 = ones, x_{t+1} = S[:, idx_t] broadcast over columns
  =>  x @ w_ih.T == outer(s_col, ws), ws = w_ih.sum(axis=1)
  Per step: z = h @ w_hh.T + outer + bias; gates -> c,h; e = mean_rows(h);
  hid = relu(pre + e @ Wb.T + ab1); scores = hid @ w2a.T; logp = log_softmax;
  idx = argmax.

Sharding: data-parallel over 4096 train rows (512/core) for the LSTM; the
setup-time `pre` projection is sharded over ANTES (each core computes the full
r-contraction for its 256-ante slice -> one AllGather, no ReduceScatter).
Per step one tiny AllGather of per-core av partials. Attention MLP, softmax
and argmax replicated on every core.

Per-step structure (vs the earlier baseline):
- bias folded into the gate activations (per-(gate,e-chunk) ACT bias), and
  ab1 folded into pre at setup; the rank-1 x@w_ih.T outer product is applied
  by scalar_tensor_tensor on vector/gpsimd reading the closed w_hh psums
  (PE freed from z_outer work after the argmax).
- s_col broadcast to 128 partitions via a K=1 ones matmul (PE), emitted
  BEFORE gate-o's w_hh matmuls to keep the psum-slot rotation acyclic.
- cell state kept in fp16; h and its row-sum produced by one fused
  tensor_tensor_reduce per e-chunk.
- av partials [1,256] AllGathered; the 8x256 block is summed via two
  transpose-DMAs + reduce (no PE transpose on the critical path).
- score chunks are DMAed psum->shist directly (no scalar copies); argmax
  stays chunked MAX8/FIND_INDEX8 on vector.

Matmuls in fp16 (accumulation fp32 in PSUM). Top-2 score margin ~0.09 vs
~1e-3 fp16 error keeps the argmax chain stable.
"""

import os
import sys
import numpy as np

sys.path.insert(0, "/opt/trn_rl_repo")

NCORES = 8
N_TRAIN, N_FEAT, N_HID, ENC = 4096, 64, 256, 256
N_ANTES, ATT_H, MAX_LEN = 2048, 256, 16
R = N_TRAIN // NCORES  # 512 rows per core
ASL = N_ANTES // NCORES  # 256 antes per core (pre sharding)
G4 = 4 * ENC  # 1024
NKR = N_TRAIN // 128  # 32 k-chunks for the pre contraction


def build_nc():
    import concourse.bass as bass
    import concourse.bacc as bacc
    import concourse.tile as tile
    from concourse import mybir
    from contextlib import ExitStack

    f32 = mybir.dt.float32
    f16 = mybir.dt.float16
    u32 = mybir.dt.uint32
    AF = mybir.ActivationFunctionType
    ALU = mybir.AluOpType
    AX = mybir.AxisListType

    nc = bacc.Bacc(num_devices=NCORES)

    # ---- kernel I/O ----
    ctxT_d = nc.dram_tensor("ctxT", [N_FEAT, R], f16, kind="ExternalInput")
    ew1T_d = nc.dram_tensor("enc_w1T", [N_FEAT, N_HID], f16, kind="ExternalInput")
    eb1_d = nc.dram_tensor("enc_b1p", [128, 2], f32, kind="ExternalInput")
    ew2T_d = nc.dram_tensor("enc_w2T", [128, 2, ENC], f16, kind="ExternalInput")
    eb2_d = nc.dram_tensor("enc_b2p", [128, 2], f32, kind="ExternalInput")
    whhT_d = nc.dram_tensor("w_hhT", [128, 2, G4], f16, kind="ExternalInput")
    wsp_d = nc.dram_tensor("wsp", [128, 8], f32, kind="ExternalInput")
    wsr_d = nc.dram_tensor("wsr", [2, G4], f16, kind="ExternalInput")
    biasp_d = nc.dram_tensor("biasp", [128, 8], f32, kind="ExternalInput")
    WbT_d = nc.dram_tensor("WbT", [128, 2, ATT_H], f16, kind="ExternalInput")
    ab1_d = nc.dram_tensor("ab1p", [128, 2], f32, kind="ExternalInput")
    w2p_d = nc.dram_tensor("w2p", [128, 2], f16, kind="ExternalInput")
    # S_shard.T plus a trailing all-ones row (indirect DMA needs >=2 offsets;
    # the second offset pins row N_ANTES)
    S_T_d = nc.dram_tensor("S_T", [N_ANTES + 1, R], f16, kind="ExternalInput")
    offi_d = nc.dram_tensor("off_init", [2, 1], u32, kind="ExternalInput")
    onesz_d = nc.dram_tensor("onesz", [2, 128], f16, kind="ExternalInput")
    # pre inputs: full-row S slice + full Wa (ante-sharded pre)
    Spre_d = nc.dram_tensor("Spre", [128, NKR, ASL], f16, kind="ExternalInput")
    WaT_d = nc.dram_tensor("WaT", [128, NKR, ATT_H], f16, kind="ExternalInput")
    out_d = nc.dram_tensor("out", [MAX_LEN, N_ANTES], f32, kind="ExternalOutput")

    # internal DRAM for collectives
    cc_av_in = nc.dram_tensor("cc_av_in", [1, ATT_H], f16)
    cc_av_out = nc.dram_tensor("cc_av_out", [NCORES, ATT_H], f16, addr_space="Shared")
    id8_d = nc.dram_tensor("ident8", [8, 8], f16, kind="ExternalInput")
    cc_pre_in = nc.dram_tensor("cc_pre_in", [ATT_H, ASL], f16)
    cc_pre_out = nc.dram_tensor(
        "cc_pre_out", [ATT_H * NCORES, ASL], f16, addr_space="Shared"
    )
    groups = [list(range(NCORES))]

    with tile.TileContext(nc, num_cores=NCORES) as tc, ExitStack() as ctx:
        const = ctx.enter_context(tc.tile_pool(name="const", bufs=1))
        state = ctx.enter_context(tc.tile_pool(name="state", bufs=1))
        work = ctx.enter_context(tc.tile_pool(name="work", bufs=2))
        zpool = ctx.enter_context(tc.tile_pool(name="zpool", bufs=3, space="PSUM"))
        apool = ctx.enter_context(tc.tile_pool(name="apool", bufs=2, space="PSUM"))

        # ---- load constants (small first; the big pre operands last) ----
        def load(shape, dram, tag, dt=f32):
            t = const.tile(shape, dt, tag=tag)
            nc.sync.dma_start(out=t[:], in_=dram[:])
            return t

        ctxT = load([N_FEAT, R], ctxT_d, "ctxT", f16)
        ew1T = load([N_FEAT, N_HID], ew1T_d, "ew1T", f16)
        eb1 = load([128, 2], eb1_d, "eb1")
        ew2T = load([128, 2, ENC], ew2T_d, "ew2T", f16)
        eb2 = load([128, 2], eb2_d, "eb2")
        whhT = load([128, 2, G4], whhT_d, "whhT", f16)
        wsp = load([128, 8], wsp_d, "wsp")
        wsr = load([2, G4], wsr_d, "wsr", f16)
        biasp = load([128, 8], biasp_d, "biasp")
        WbT = load([128, 2, ATT_H], WbT_d, "WbT", f16)
        ab1 = load([128, 2], ab1_d, "ab1")
        w2p = load([128, 2], w2p_d, "w2p", f16)
        id8 = load([8, 8], id8_d, "id8", f16)
        ones1 = load([2, 128], onesz_d, "onesz", f16)

        # ---- persistent state ----
        hT = state.tile([128, 2, R], f16)  # h transposed: [enc(128x2), rows]
        cT = state.tile([128, 2, R], f16)
        s_col2 = state.tile([2, R], f16)  # row0 = S_shard[:, idx]

        pre0 = state.tile([128, N_ANTES], f16)  # (pre+ab1).T rows 0:128
        pre1 = state.tile([128, N_ANTES], f16)  # rows 128:256
        shist = state.tile([MAX_LEN, N_ANTES], f16)  # raw scores per step
        hsum = state.tile([128, 2], f32)
        hsum16 = state.tile([128, 2], f16)
        avb = state.tile([128, 2], f32)
        off2 = state.tile([2, 1], u32)
        cmax = state.tile([1, 4, 8], f16)  # per-512-chunk top-8 score values
        cmif = state.tile([1, 4], f32)  # per-chunk argmax as f32
        gmx = state.tile([1, 1], f32)
        eqm = state.tile([1, 4], f32)
        msk = state.tile([1, 4], f32)
        idxf = state.tile([1, 1], f32)
        scs = state.tile([1, N_ANTES], f16)  # current step's scores (partition 0)

        nc.vector.memset(cT[:], 0.0)

        nc.sync.dma_start(out=off2[:], in_=offi_d[:])

        # ---- phi = encoder(context) -> h_0 (transposed layout) ----
        a1_tiles = []
        for m in range(2):
            p = apool.tile([128, R], f32, tag="abank")
            nc.tensor.matmul(
                p[:], ew1T[:, m * 128 : (m + 1) * 128], ctxT[:],
                start=True, stop=True,
            )
            a1 = work.tile([128, R], f16, tag="a1")
            nc.scalar.activation(a1[:], p[:], AF.Relu, bias=eb1[:, m : m + 1])
            a1_tiles.append(a1)
        for m in range(2):
            p = apool.tile([128, R], f32, tag="abank")
            for k in range(2):
                nc.tensor.matmul(
                    p[:], ew2T[:, k, m * 128 : (m + 1) * 128], a1_tiles[k][:],
                    start=(k == 0), stop=(k == 1),
                )
            nc.scalar.activation(
                hT[:, m, :], p[:], AF.Identity, bias=eb2[:, m : m + 1]
            )

        # ---- per-step helpers ----
        # zin[g] holds z(e0) = w_hh@h + ws (x) s_col for gate g, fp16 SBUF;
        # the e1 half keeps the outer product as a K=1 PE matmul into the
        # open psum group (gpsimd cannot touch PSUM, vector would saturate).
        zin = [None] * 4
        zp = [None] * 4

        def kmm(g):
            # e0 group closed (consumed by the vector STT); e1 left open for
            # the K=1 outer matmul after the argmax.
            p = zpool.tile([128, 2, 512], f32, tag="zbank")
            for e in range(2):
                m = 2 * g + e
                for k in range(2):
                    nc.tensor.matmul(
                        p[:, e, :],
                        whhT[:, k, m * 128 : (m + 1) * 128],
                        hT[:, k, :],
                        start=(k == 0), stop=False,
                    )
            return p

        def outer_e1(g):
            for e in range(2):
                m = 2 * g + e
                nc.tensor.matmul(
                    zp[g][:, e, :], wsr[0:2, m * 128 : (m + 1) * 128],
                    s_col2[0:2, :], start=False, stop=True,
                )


        GORDER = [0, 2, 1, 3]  # i, g, f, o: ig-mul inputs first, o last

        def acts_and_cell():
            sig = [None] * 4
            for g in GORDER:
                sig[g] = work.tile(
                    [128, 2, 512], f16, tag="g%d" % g, name="sig%d" % g
                )
            for g in GORDER:
                func = AF.Tanh if g == 2 else AF.Sigmoid
                for e in range(2):
                    nc.scalar.activation(
                        sig[g][:, e, :], zp[g][:, e, :], func,
                        bias=biasp[:, 2 * g + e : 2 * g + e + 1],
                    )
            ig = work.tile([128, 2, 512], f16, tag="ig")
            nc.vector.tensor_mul(ig[:], sig[0][:], sig[2][:])
            cf = work.tile([128, 2, 512], f16, tag="cf")
            nc.vector.tensor_mul(cf[:], cT[:], sig[1][:])
            nc.vector.tensor_add(cT[:], ig[:], cf[:])
            th = work.tile([128, 2, 512], f16, tag="th")
            nc.scalar.activation(th[:], cT[:], AF.Tanh)
            nc.vector.tensor_mul(hT[:], sig[3][:], th[:])
            nc.vector.reduce_sum(hsum[:], hT[:], axis=AX.X)
            nc.vector.tensor_copy(hsum16[:], hsum[:])

        def av_exchange():
            pav = apool.tile([1, ATT_H], f32, tag="abank")
            nc.tensor.matmul(
                pav[:], hsum16[:, 0:1], WbT[:, 0, :], start=True, stop=False
            )
            nc.tensor.matmul(
                pav[:], hsum16[:, 1:2], WbT[:, 1, :], start=False, stop=True
            )
            avp16 = work.tile([1, ATT_H], f16, tag="avp")
            nc.vector.tensor_copy(avp16[:], pav[:])
            nc.sync.dma_start(out=cc_av_in[:], in_=avp16[:])
            nc.gpsimd.collective_compute(
                "AllGather", ALU.bypass, replica_groups=groups,
                ins=[cc_av_in[:]], outs=[cc_av_out[:]],
            )

        # ---- step 0 z: x_0 = ones (s_colb / s_col2 pre-set) ----
        nc.vector.memset(s_col2[:], 1.0)
        for g in GORDER:
            zp[g] = kmm(g)
        for g in GORDER:
            outer_e1(g)
        acts_and_cell()
        av_exchange()

        # prefill next step's w_hh matmuls for gates i,g,f under the AG
        for g in GORDER[:3]:
            zp[g] = kmm(g)

        # ---- pre slice: full r-contraction for this core's 256 antes ----
        with tc.tile_pool(name="spool", bufs=1) as spool:
            Spre_sb = spool.tile([128, NKR, ASL], f16)
            nc.sync.dma_start(out=Spre_sb[:], in_=Spre_d[:])
            WaT_sb = spool.tile([128, NKR, ATT_H], f16)
            nc.sync.dma_start(out=WaT_sb[:], in_=WaT_d[:])
            for m in range(2):
                p = apool.tile([128, ASL], f32, tag="abank")
                for k in range(NKR):
                    nc.tensor.matmul(
                        p[:],
                        WaT_sb[:, k, m * 128 : (m + 1) * 128],
                        Spre_sb[:, k, :],
                        start=(k == 0), stop=(k == NKR - 1),
                    )
                prep = work.tile([128, ASL], f16, tag="prep%d" % m)
                # fold the attention bias ab1 into pre here
                nc.scalar.activation(
                    prep[:], p[:], AF.Identity, bias=ab1[:, m : m + 1]
                )
                nc.sync.dma_start(
                    out=cc_pre_in[m * 128 : (m + 1) * 128, :], in_=prep[:]
                )
        nc.gpsimd.collective_compute(
            "AllGather", ALU.bypass, replica_groups=groups,
            ins=[cc_pre_in[:]], outs=[cc_pre_out[:]],
        )
        for c in range(NCORES):
            base = c * ATT_H
            nc.sync.dma_start(
                out=pre0[:, c * ASL : (c + 1) * ASL],
                in_=cc_pre_out[base : base + 128, :],
            )
            nc.sync.dma_start(
                out=pre1[:, c * ASL : (c + 1) * ASL],
                in_=cc_pre_out[base + 128 : base + ATT_H, :],
            )

        # ---- 16 attention + LSTM steps ----
        for t in range(MAX_LEN):
            last = t == MAX_LEN - 1

            # av_all [8,256] -> avb [128,2] via PE transposes + reduce
            av_all = work.tile([NCORES, ATT_H], f16, tag="avall")
            nc.sync.dma_start(out=av_all[:], in_=cc_av_out[:])
            for e in range(2):
                pt = apool.tile([128, 8], f16, tag="abank", name="pt%d" % e)
                nc.tensor.transpose(
                    pt[:], av_all[0:NCORES, e * 128 : (e + 1) * 128], id8[:]
                )
                nc.vector.reduce_sum(avb[:, e : e + 1], pt[:], axis=AX.X)

            # hid = relu(pre + avb) ; scores = w2 . hid, chunked by 512 antes
            hid0 = work.tile([128, N_ANTES], f16, tag="hid0")
            hid1 = work.tile([128, N_ANTES], f16, tag="hid1")
            for n in range(4):
                sl = slice(n * 512, (n + 1) * 512)
                nc.scalar.activation(
                    hid0[:, sl], pre0[:, sl], AF.Relu, bias=avb[:, 0:1]
                )
                nc.vector.tensor_scalar(
                    hid1[:, sl], pre1[:, sl], avb[:, 1:2], 0.0,
                    op0=ALU.add, op1=ALU.max,
                )
                ps = apool.tile([1, 512], f32, tag="abank")
                nc.tensor.matmul(
                    ps[:], w2p[:, 0:1], hid0[:, sl], start=True, stop=False
                )
                nc.tensor.matmul(
                    ps[:], w2p[:, 1:2], hid1[:, sl], start=False, stop=True
                )
                # engine copy (gpsimd cannot read PSUM, DMA cannot either)
                nc.scalar.copy(scs[0:1, sl], ps[:])
                if not last:
                    nc.vector.max(cmax[:, n, :], scs[0:1, sl])
                    mi = work.tile([1, 8], u32, tag="mi%d" % n)
                    nc.vector.max_index(mi[:], cmax[:, n, :], scs[0:1, sl])
                    nc.vector.tensor_scalar(
                        cmif[0:1, n : n + 1], mi[0:1, 0:1], float(512 * n),
                        None, op0=ALU.add,
                    )
            nc.sync.dma_start(out=shist[t : t + 1, :], in_=scs[:])

            if last:
                break

            # combine the 4 chunk argmaxes; final copy on gpsimd (gather queue)
            nc.vector.reduce_max(gmx[:], cmax[0:1, :, 0], axis=AX.X)
            nc.vector.tensor_scalar(
                eqm[:], cmax[0:1, :, 0], gmx[0:1, 0:1], None, op0=ALU.is_equal
            )
            nc.vector.tensor_mul(msk[:], eqm[:], cmif[:])
            nc.vector.reduce_max(idxf[:], msk[:], axis=AX.X)
            nc.vector.tensor_copy(off2[0:1, 0:1], idxf[:])
            nc.gpsimd.indirect_dma_start(
                out=s_col2[:],
                out_offset=None,
                in_=S_T_d[:],
                in_offset=bass.IndirectOffsetOnAxis(ap=off2[:, 0:1], axis=0),
            )
            for g in GORDER[:3]:
                outer_e1(g)
            zp[3] = kmm(3)
            outer_e1(3)
            acts_and_cell()
            av_exchange()
            if t < MAX_LEN - 2:
                for g in GORDER[:3]:
                    zp[g] = kmm(g)

        # ---- batched log_softmax over all 16 steps ----
        mx = state.tile([MAX_LEN, 1], f32)
        nc.vector.reduce_max(mx[:], shist[:], axis=AX.X)
        negm = state.tile([MAX_LEN, 1], f32)
        nc.vector.tensor_scalar_mul(negm[:], mx[:], -1.0)
        etile = work.tile([MAX_LEN, N_ANTES], f32, tag="etile")
        sume = state.tile([MAX_LEN, 1], f32)
        nc.scalar.activation(
            etile[:], shist[:], AF.Exp, bias=negm[:], accum_out=sume[:]
        )
        lsum = state.tile([MAX_LEN, 1], f32)
        nc.scalar.activation(lsum[:], sume[:], AF.Ln)
        offc = state.tile([MAX_LEN, 1], f32)
        nc.vector.tensor_add(offc[:], mx[:], lsum[:])
        nc.vector.tensor_scalar_mul(offc[:], offc[:], -1.0)
        logp = work.tile([MAX_LEN, N_ANTES], f32, tag="logp")
        nc.scalar.activation(logp[:], shist[:], AF.Identity, bias=offc[:])
        nc.sync.dma_start(out=out_d[:], in_=logp[:])

    nc.compile()
    return nc


def pack2(a):
    """[nk*128, X] -> [128, nk, X] so that a[k*128+p, x] == out[p, k, x]."""
    rows, X = a.shape
    nk = rows // 128
    return np.ascontiguousarray(a.reshape(nk, 128, X).transpose(1, 0, 2))


def colpack(v):
    """[nk*128] -> [128, nk] per-partition bias layout."""
    n = v.shape[0]
    nk = n // 128
    return np.ascontiguousarray(v.reshape(nk, 128).T)


def make_in_maps(inputs):
    f32 = np.float32
    f16 = np.float16
    context = np.ascontiguousarray(np.asarray(inputs["context"], f32))
    S = np.ascontiguousarray(np.asarray(inputs["S"], f32))
    enc_w1 = np.asarray(inputs["enc_w1"], f32)
    enc_b1 = np.asarray(inputs["enc_b1"], f32)
    enc_w2 = np.asarray(inputs["enc_w2"], f32)
    enc_b2 = np.asarray(inputs["enc_b2"], f32)
    w_ih = np.asarray(inputs["w_ih"], f32)
    w_hh = np.asarray(inputs["w_hh"], f32)
    b_ih = np.asarray(inputs["b_ih"], f32)
    b_hh = np.asarray(inputs["b_hh"], f32)
    att_w1 = np.asarray(inputs["att_w1"], f32)
    att_b1 = np.asarray(inputs["att_b1"], f32)
    att_w2 = np.asarray(inputs["att_w2"], f32)

    ws = w_ih.sum(axis=1, dtype=np.float64).astype(f32)

    shared = {
        "enc_w1T": np.ascontiguousarray(enc_w1.T).astype(f16),
        "enc_b1p": colpack(enc_b1),
        "enc_w2T": pack2(np.ascontiguousarray(enc_w2.T)).astype(f16),
        "enc_b2p": colpack(enc_b2),
        "w_hhT": pack2(np.ascontiguousarray(w_hh.T)).astype(f16),
        "wsp": colpack(ws),
        "wsr": np.stack([ws, np.zeros_like(ws)], axis=0).astype(f16),
        "biasp": colpack(b_ih + b_hh),
        "WbT": pack2(np.ascontiguousarray(att_w1[:, N_TRAIN:].T) / N_TRAIN).astype(
            f16
        ),
        "ab1p": colpack(att_b1),
        "w2p": colpack(att_w2[0]).astype(f16),
        "WaT": pack2(np.ascontiguousarray(att_w1[:, :N_TRAIN].T)).astype(f16),
        "off_init": np.array([[0], [N_ANTES]], dtype=np.uint32),
        "ident8": np.eye(8, dtype=f16),
        "onesz": np.stack([np.ones(128, f16), np.zeros(128, f16)]),
    }
    in_maps = []
    for c in range(NCORES):
        rows = slice(c * R, (c + 1) * R)
        antes = slice(c * ASL, (c + 1) * ASL)
        m = dict(shared)
        m["ctxT"] = np.ascontiguousarray(context[rows].T).astype(f16)
        st = np.ascontiguousarray(S[rows].T).astype(f16)
        m["S_T"] = np.concatenate([st, np.ones((1, R), f16)], axis=0)
        m["Spre"] = pack2(np.ascontiguousarray(S[:, antes])).astype(f16)
        in_maps.append(m)
    return in_maps


_NC = None


def kernel(**inputs):
    global _NC
    from concourse.bass_utils import run_bass_kernel_spmd

    if _NC is None:
        _NC = build_nc()
    in_maps = make_in_maps(inputs)
    trace = bool(int(os.environ.get("KERNEL_TRACE", "0")))
    if trace:
        try:  # NTFF profiling needs the antenv.axon_hooks shim
            import profile_shim

            profile_shim.install()
        except Exception:
            trace = False
    res = run_bass_kernel_spmd(
        _NC, in_maps, list(range(NCORES)), trace=trace,
    )
    out = np.asarray(res.results[0]["out"], np.float32)
    if res.exec_time_ns is not None:
        print(f"HW exec time: {res.exec_time_ns} ns")
    return out
